# revision 7
# baseline (speedup 1.0000x reference)
"""Trainium2 Bass kernel v4 for nn_AuxiliaryClustering (segment_reduce).

Per core: 31 groups of 4096 rows. Fully software-pipelined:
  iteration g emits   gp:  load a(g), z(g); accA += a(g-1)  [SWDGE]
                      DVE: rmax(g), is_eq(g), d2red(g-1)
                      SP:  XBAR transpose oh(g)
                      ACT: square(g-1) x2, sqrt(g-1)
                      PE:  Z(g,0) | G(g-1,h0) G(g-1,h1) papp(g-2) | Z(g,1)
so every PE matmul's inputs are >= 1 group old when it issues (no
weight-load stalls, PE stays at high clock). Colsum of A runs entirely
on the DMA engines (SBUF->SBUF accumulate-cast into an f32 buffer).
A is fp16 with a unique per-(row,col) mantissa tie-code (exact single
argmax, per-column-unbiased colsum). PSUM: 3 csel bufs (2 banks each)
+ papp accumulator (2 banks).
"""

import os
from contextlib import ExitStack

import numpy as np

import concourse.bass as bass
import concourse.bacc as bacc
import concourse.tile as tile
from concourse import mybir
from concourse.bass_utils import run_bass_kernel_spmd

F32 = mybir.dt.float32
F16 = mybir.dt.float16
AX = mybir.AxisListType
OP = mybir.AluOpType

B, D, K = 1000000, 64, 64
NCORES = 8
P = 128
R = 32
GROUP_ROWS = P * R                                  # 4096
SHARD = B // NCORES                                 # 125000
NG = (SHARD + GROUP_ROWS - 1) // GROUP_ROWS         # 31
PAD_SHARD = NG * GROUP_ROWS                         # 126976
NPAD = PAD_SHARD - SHARD                            # 1976
NCH = R * D // P                                    # 16 chunks per group

EPS = 1e-08
WEIGHT = 0.1


def build_nc(ng: int = NG, pad_shard: int = PAD_SHARD):
    nc = bacc.Bacc("TRN2", target_bir_lowering=False, debug=False)

    a_d = nc.dram_tensor("a", [pad_shard, K], F16, kind="ExternalInput").ap()
    z_d = nc.dram_tensor("z", [pad_shard, D], F16, kind="ExternalInput").ap()
    cbf_d = nc.dram_tensor("cbf", [P, P], F16, kind="ExternalInput").ap()
    ident_d = nc.dram_tensor("ident", [P, P], F16, kind="ExternalInput").ap()
    c_d = nc.dram_tensor("c", [K, D], F32, kind="ExternalInput").ap()
    ct_d = nc.dram_tensor("ct", [D, K], F32, kind="ExternalInput").ap()
    mask_d = nc.dram_tensor("mask", [K, K], F32, kind="ExternalInput").ap()

    outp_d = nc.dram_tensor("out_papp", [R, P * NCH // 2], F32,
                            kind="ExternalOutput").ap()
    outm_d = nc.dram_tensor("out_misc", [P, K + 1], F32,
                            kind="ExternalOutput").ap()

    a_4d = a_d.rearrange("(t p r) d -> t p r d", p=P, r=R)
    z_4d = z_d.rearrange("(t p r) d -> t p r d", p=P, r=R)

    with tile.TileContext(nc) as tc, ExitStack() as ctx:
        iop = ctx.enter_context(tc.tile_pool(name="io", bufs=3))
        ohp = ctx.enter_context(tc.tile_pool(name="oh", bufs=4))
        ohtp = ctx.enter_context(tc.tile_pool(name="oht", bufs=3))
        wp = ctx.enter_context(tc.tile_pool(name="work", bufs=2))
        cp = ctx.enter_context(tc.tile_pool(name="const", bufs=1))
        ps_cs = ctx.enter_context(tc.tile_pool(name="ps_cs", bufs=3, space="PSUM"))
        ps_acc = ctx.enter_context(tc.tile_pool(name="ps_acc", bufs=1, space="PSUM"))

        # --- constants ---
        cbf_t = cp.tile([P, P], F16)
        nc.sync.dma_start(out=cbf_t[:], in_=cbf_d[:])
        ident_t = cp.tile([P, P], F16)
        nc.sync.dma_start(out=ident_t[:], in_=ident_d[:])
        ones_t = cp.tile([P, 1], F32)
        nc.vector.memset(ones_t[:], 1.0)
        accA_t = cp.tile([P, R * K], F32)
        nc.vector.memset(accA_t[:], 0.0)

        # =====================  separation loss (tiny, one-shot)  ==========
        sep_t = cp.tile([K, 1], F32)
        c_t = cp.tile([K, D], F32)
        nc.sync.dma_start(out=c_t[:], in_=c_d[:])
        ct_t = cp.tile([D, K], F32)
        nc.sync.dma_start(out=ct_t[:], in_=ct_d[:])
        mask_t = cp.tile([K, K], F32)
        nc.sync.dma_start(out=mask_t[:], in_=mask_d[:])

        csq_t = cp.tile([K, D], F32)
        nc.vector.tensor_tensor(out=csq_t[:], in0=c_t[:], in1=c_t[:], op=OP.mult)
        csqc_t = cp.tile([K, 1], F32)
        nc.vector.reduce_sum(csqc_t[:], csq_t[:], axis=AX.X)
        ctsq_t = cp.tile([D, K], F32)
        nc.vector.tensor_tensor(out=ctsq_t[:], in0=ct_t[:], in1=ct_t[:], op=OP.mult)

        acc_ps = ps_acc.tile([K, P * NCH // 2], F32, tag="acc")
        g_ps = ps_acc.tile([K, K], F32, tag="acc")
        nc.tensor.matmul(g_ps[:], ct_t[:], ct_t[:], start=True, stop=True)
        row_ps = ps_acc.tile([1, K], F32, tag="acc")
        nc.tensor.matmul(row_ps[:], ones_t[0:D, :], ctsq_t[:], start=True, stop=True)

        t1_t = cp.tile([K, K], F32)
        nc.scalar.activation(
            out=t1_t[:], in_=g_ps[:], func=mybir.ActivationFunctionType.Identity,
            bias=csqc_t[:], scale=-2.0,
        )
        csqr_sb = cp.tile([1, K], F32)
        nc.scalar.copy(out=csqr_sb[:], in_=row_ps[:])
        csqr_b = cp.tile([K, K], F32)
        nc.gpsimd.partition_broadcast(csqr_b[:], csqr_sb[:])
        d2m_t = cp.tile([K, K], F32)
        nc.vector.tensor_tensor(out=d2m_t[:], in0=t1_t[:], in1=csqr_b[:], op=OP.add)
        nc.vector.tensor_scalar_max(out=d2m_t[:], in0=d2m_t[:], scalar1=0.0)
        dm_t = cp.tile([K, K], F32)
        nc.scalar.sqrt(dm_t[:], d2m_t[:])
        nc.vector.tensor_tensor(out=dm_t[:], in0=dm_t[:], in1=mask_t[:], op=OP.mult)
        nc.vector.reduce_sum(sep_t[:], dm_t[:], axis=AX.X)

        papp_ps = acc_ps[0:R, :]

        # ---- pipeline stage emitters ----
        st = {}   # per-group state: a2d, z2d, oh2d, ohT, csel halves, sq, do

        def stage_load(g):
            a_t = iop.tile([P, R, K], F16, tag="a")
            nc.gpsimd.dma_start(out=a_t[:], in_=a_4d[g])
            z_t = iop.tile([P, R, D], F16, tag="z")
            nc.gpsimd.dma_start(out=z_t[:], in_=z_4d[g])
            st[g] = {
                "a2d": a_t[:].rearrange("p r d -> p (r d)"),
                "z2d": z_t[:].rearrange("p r d -> p (r d)"),
                "a_t": a_t, "z_t": z_t,
            }

        def stage_accA(g):
            nc.gpsimd.dma_start(out=accA_t[:], in_=st[g]["a2d"], accum_op=OP.add)

        def stage_onehot(g):
            s = st[g]
            a_t = s["a_t"]
            m_t = wp.tile([P, R], F16, tag="m")
            nc.vector.reduce_max(m_t[:], a_t[:], axis=AX.X)
            oh_t = ohp.tile([P, R, K], F16, tag="oh")
            m3 = m_t[:].rearrange("p (r one) -> p r one", one=1)
            nc.vector.tensor_tensor(
                out=oh_t[:], in0=a_t[:],
                in1=m3.broadcast_to([P, R, K]), op=OP.is_equal,
            )
            s["oh2d"] = oh_t[:].rearrange("p r d -> p (r d)")

        def stage_xbar(g):
            s = st[g]
            ohT_t = ohtp.tile([P, NCH, P], F16, tag="ohT")
            nc.sync.dma_start(out=ohT_t[:], in_=s["oh2d"], transpose=True)
            s["ohT"] = ohT_t

        def stage_z(g, h):
            s = st[g]
            csel_ps = ps_cs.tile([P, NCH // 2, P], F32, tag="csel")
            s.setdefault("csel", {})[h] = csel_ps
            csel2d = csel_ps[:].rearrange("p c q -> p (c q)")
            for q in range(2):
                nc.tensor.matmul(
                    csel2d[:, q * 512:(q + 1) * 512],
                    ident_t[:],
                    s["z2d"][:, h * 1024 + q * 512:h * 1024 + (q + 1) * 512],
                    start=True, stop=False, skip_group_check=True,
                )

        def stage_gather(g, h):
            s = st[g]
            for j in range(NCH // 2):
                nc.tensor.matmul(
                    s["csel"][h][:, j, :],
                    s["ohT"][:, h * 8 + j, :], cbf_t[:],
                    start=False, stop=(j == NCH // 2 - 1),
                    skip_group_check=True,
                )

        def stage_square(g, h):
            s = st[g]
            if "sq" not in s:
                sq_t = wp.tile([P, R, D], F16, tag="sq")
                s["sq"] = sq_t
            sq2d = s["sq"][:].rearrange("p r d -> p (r d)")
            nc.scalar.square(
                sq2d[:, h * 1024:(h + 1) * 1024],
                s["csel"][h][:].rearrange("p c q -> p (c q)"),
            )

        def stage_dist(g):
            s = st[g]
            d2_t = wp.tile([P, R], F16, tag="d2")
            with nc.allow_low_precision("fp16 dist^2 is fine for this loss"):
                nc.vector.reduce_sum(d2_t[:], s["sq"][:], axis=AX.X)
            do_t = wp.tile([P, R, 2], F16, tag="do")
            nc.vector.memset(do_t[:, :, 1:2], 1.0)
            nc.scalar.sqrt(
                do_t[:, :, 0:1],
                d2_t[:].rearrange("p (r one) -> p r one", one=1),
            )
            s["do2d"] = do_t[:].rearrange("p r c -> p (r c)")

        def stage_papp(g, ng_):
            s = st[g]
            for h in range(2):
                for q in range(2):
                    nc.tensor.matmul(
                        papp_ps[:, q * 512:(q + 1) * 512],
                        s["do2d"][:, h * 32:(h + 1) * 32],
                        s["oh2d"][:, h * 1024 + q * 512:h * 1024 + (q + 1) * 512],
                        start=(g == 0 and h == 0), stop=(g == ng_ - 1 and h == 1),
                        skip_group_check=True,
                    )
            del st[g]

        # =====================  main pipelined loop  =======================
        # lags: gathers/squares -1, dist -2, papp -3 -- each stage's inputs
        # are ready well before its engine reaches it (no exposed latency)
        for g in range(ng):
            stage_load(g)
            if g >= 1:
                stage_accA(g - 1)
            stage_onehot(g)
            stage_xbar(g)
            stage_z(g, 0)
            if g >= 1:
                stage_gather(g - 1, 0)
                stage_square(g - 1, 0)
                stage_gather(g - 1, 1)
                stage_square(g - 1, 1)
            if g >= 2:
                stage_dist(g - 2)
            if g >= 3:
                stage_papp(g - 3, ng)
            stage_z(g, 1)

        # drain
        stage_accA(ng - 1)
        stage_gather(ng - 1, 0)
        stage_square(ng - 1, 0)
        stage_gather(ng - 1, 1)
        stage_square(ng - 1, 1)
        if ng >= 2:
            stage_dist(ng - 2)
        stage_dist(ng - 1)
        for gp_ in range(max(0, ng - 3), ng):
            stage_papp(gp_, ng)

        # =====================  epilogue  ==================================
        colk_t = cp.tile([P, K], F32)
        acc_kr = accA_t[:].rearrange("p (r d) -> p d r", d=K)
        nc.vector.reduce_sum(colk_t[:], acc_kr, axis=AX.X)

        misc_t = cp.tile([P, K + 1], F32)
        nc.vector.memset(misc_t[:], 0.0)
        nc.vector.tensor_copy(out=misc_t[:, 0:K], in_=colk_t[:])
        nc.vector.tensor_copy(out=misc_t[0:K, K:K + 1], in_=sep_t[:])
        nc.sync.dma_start(out=outm_d[:], in_=misc_t[:])

        papp_sb = cp.tile([R, P * NCH // 2], F32)
        nc.scalar.copy(out=papp_sb[:], in_=papp_ps)
        nc.sync.dma_start(out=outp_d[:], in_=papp_sb[:])

    nc.finalize()
    return nc


_NC_CACHE = {}


def _get_nc():
    if "nc" not in _NC_CACHE:
        _NC_CACHE["nc"] = build_nc()
    return _NC_CACHE["nc"]


def _tweak_a(a: np.ndarray) -> np.ndarray:
    """fp16 bits with low-6 mantissa replaced by a unique per-(row,col)
    tie-code. Truncate + uniform code keeps the colsum per-column unbiased."""
    au = a.astype(np.float16).view(np.uint16)
    n, k = au.shape
    code = (63 - np.arange(k, dtype=np.uint16))[None, :] ^ \
        (np.arange(n, dtype=np.uint64)[:, None] & 63).astype(np.uint16)
    return ((au & 0xFFC0) | code).view(np.float16)


def make_inputs(a_s, z_s, c):
    cbf1 = c.astype(np.float16)
    cbf = np.zeros((P, P), dtype=np.float16)
    cbf[:K, :D] = -cbf1
    cbf[K:, D:] = -cbf1
    return {
        "a": a_s, "z": z_s, "cbf": cbf,
        "ident": np.eye(P, dtype=np.float32).astype(np.float16),
        "c": c,
        "ct": np.ascontiguousarray(c.T),
        "mask": (1.0 - np.eye(K, dtype=np.float32)),
    }


def kernel(latent_z, cluster_assignments, cluster_centers):
    z = np.asarray(latent_z, dtype=np.float32)
    a = np.ascontiguousarray(np.asarray(cluster_assignments, dtype=np.float32))
    c = np.ascontiguousarray(np.asarray(cluster_centers, dtype=np.float32))

    at = _tweak_a(a)
    zf = z.astype(np.float16)
    cf = c.astype(np.float16)

    a_pad_row = np.zeros((K,), dtype=np.float16)
    a_pad_row[0] = 1.0
    z_pad_row = cf[0]

    in_maps = []
    for core in range(NCORES):
        lo, hi = core * SHARD, (core + 1) * SHARD
        a_s = np.empty((PAD_SHARD, K), dtype=np.float16)
        z_s = np.empty((PAD_SHARD, D), dtype=np.float16)
        a_s[:SHARD] = at[lo:hi]
        z_s[:SHARD] = zf[lo:hi]
        a_s[SHARD:] = a_pad_row
        z_s[SHARD:] = z_pad_row
        in_maps.append(make_inputs(a_s, z_s, c))

    nc = _get_nc()
    trace = bool(int(os.environ.get("KERNEL_PROFILE", "0")))
    res = run_bass_kernel_spmd(
        nc, in_maps, list(range(NCORES)), trace=trace, trace_cores=[0],
    )
    if trace:
        _NC_CACHE["exec_time_ns"] = res.exec_time_ns
        print(f"HW exec time: {res.exec_time_ns} ns")

    papp = np.stack([r["out_papp"] for r in res.results]).astype(np.float64)
    misc = np.stack([r["out_misc"] for r in res.results]).astype(np.float64)

    dist_sum = np.zeros(K)
    counts = np.zeros(K)
    for s in range(16):
        cols = 64 * s + np.arange(K)
        dist_sum += papp[:, 2 * s, cols].sum(axis=0)
        counts += papp[:, 2 * s + 1, cols].sum(axis=0)
    colsum = misc[:, :, 0:K].sum(axis=(0, 1))
    sep_rowsum = misc[0, :K, K]

    counts[0] -= NCORES * NPAD
    colsum[0] -= NCORES * NPAD

    probs = colsum / B
    balance = float(np.sum((1.0 / K) * (np.log(1.0 / K) - np.log(probs + EPS))))
    separation = float(-np.sum(sep_rowsum) / (K * (K - 1)))
    nonempty = counts > 0
    per_mean = dist_sum / np.maximum(counts, 1.0)
    n_nonempty = float(nonempty.sum())
    compact = float(np.sum(np.where(nonempty, per_mean, 0.0)) / max(n_nonempty, 1.0))
    aux = WEIGHT * balance + WEIGHT * separation + WEIGHT * compact
    cluster_balance = float(np.std(probs, ddof=1))

    return (
        np.float32(aux),
        np.float32(balance),
        np.float32(separation),
        np.float32(compact),
        np.float32(cluster_balance),
    )
